# revision 29
# baseline (speedup 1.0000x reference)
"""Trainium2 Bass kernel for nn_Always (sliding-window smoothed-min).

The reference "scan" is a sliding-window reduction:
    out[b, t, d] = -(1/5) * log( sum_{k=0..15} exp(-5 * x[b, t-k, d]) )
with x[b, j, d] := x[b, 0, d] for j < 0 (the h0 padding).

Strategy (pure data parallel over 8 cores; 2 batches x 2 tensors per core):
  - layout: time tiles of 256 timesteps: t = 256*J + 2*p + i with p the SBUF
    partition and (i, d) in the free dim. Two consecutive t-rows per
    partition make every DMA descriptor 512 B, lifting HBM DMA efficiency.
  - ScalarE: E = exp(-5x), in place (f32r tiles)
  - TensorE: banded-matrix matmuls compute the 16-wide window sum S.
    With 2 rows/partition the band splits into 4 (out-parity, in-parity)
    weight pairs per class: W_in[oi][ii] (within-tile), W_halo[oi][ii]
    (previous tile), W_first[oi] (t=0 padding).
  - ScalarE: ln(S) from PSUM; VectorE: * -1/5
  Scheduling structure: all 8 input DMAs are emitted first on the SP
  sequencer (so no input transfer ever queues behind an output DMA whose
  semaphore wait is still pending), activations run in pinned groups of
  4 exps / 4 lns (4 ACT table-set loads total instead of ~10), and all
  output DMAs are emitted last.
"""

import numpy as np

B, T, D = 16, 8192, 64
N_CORES = 8
B_PER_CORE = B // N_CORES  # 2
SCALE = 5.0
WIN = 16
P = 128                     # SBUF partitions
ROWS = 2                    # timesteps per partition per tile
TILE_T = P * ROWS           # 256 timesteps per tile
TILE_COLS = ROWS * D        # 128 free columns per tile
CHUNK_TILES = 16            # tiles per chunk
CHUNK_COLS = CHUNK_TILES * TILE_COLS       # 2048
CHUNKS_PER_SEQ = T // (TILE_T * CHUNK_TILES)  # 2
HALF = CHUNK_TILES // 2     # 8 tiles per psum bank
N_SEQS = 2 * B_PER_CORE     # 4 sequences per core
N_CHUNKS = N_SEQS * CHUNKS_PER_SEQ  # 8
ACT_GROUP = 4               # chunks per exp/ln activation group


def _weight_mats():
    """Returns the 10 banded matrices, concatenated [128, 1280]:
    order: W_in[0][0], W_in[0][1], W_in[1][0], W_in[1][1],
           W_halo[0][0], ..., W_halo[1][1], W_first[0], W_first[1].
    Layout convention: lhsT[p_in, p_out]; matmul computes lhsT.T @ rhs."""
    p = np.arange(P)
    mats = []
    for cls in ("in", "halo"):
        for oi in (0, 1):
            for ii in (0, 1):
                t_out = 2 * p[None, :] + oi
                t_in = 2 * p[:, None] + ii
                dd = t_out - t_in + (TILE_T if cls == "halo" else 0)
                lo = 1 if cls == "halo" else 0
                mats.append(((dd >= lo) & (dd <= WIN - 1)).astype(np.float32))
    for oi in (0, 1):
        wf = np.zeros((P, P), np.float32)
        wf[0, :] = np.maximum(WIN - 1 - (2 * p + oi), 0)
        mats.append(wf)
    return np.concatenate(mats, axis=1)


def _build_bass(mode="grouped"):
    from contextlib import ExitStack

    import concourse.bacc as bacc
    import concourse.tile as tile
    from concourse import mybir
    from concourse.tile import add_dep_helper

    f32 = mybir.dt.float32
    f32r = mybir.dt.float32r
    AF = mybir.ActivationFunctionType

    nc = bacc.Bacc(trn_type="TRN2")
    lo = nc.dram_tensor("lower", [B_PER_CORE, T, D], f32, kind="ExternalInput")
    up = nc.dram_tensor("upper", [B_PER_CORE, T, D], f32, kind="ExternalInput")
    out_lo = nc.dram_tensor("out_lower", [B_PER_CORE, T, D], f32, kind="ExternalOutput")
    out_up = nc.dram_tensor("out_upper", [B_PER_CORE, T, D], f32, kind="ExternalOutput")

    w_all_d = nc.inline_tensor(_weight_mats(), name="w_all_c")

    def view3(ap):
        return ap.rearrange("p (J i d) -> p J i d", i=ROWS, d=D)

    with tile.TileContext(nc) as tc, ExitStack() as ctx:
        consts = ctx.enter_context(tc.tile_pool(name="consts", bufs=1))
        x_pool = ctx.enter_context(tc.tile_pool(name="x", bufs=6))
        e_pool = ctx.enter_context(tc.tile_pool(name="e", bufs=N_CHUNKS))
        o_pool = ctx.enter_context(tc.tile_pool(name="o", bufs=4))
        ps_pool = ctx.enter_context(tc.tile_pool(name="ps", bufs=2, space="PSUM"))

        w_all = consts.tile([P, 10 * P], f32r)

        def w(idx):
            return w_all[:, idx * P : (idx + 1) * P]

        W_IN = lambda oi, ii: w(oi * 2 + ii)          # noqa: E731
        W_HALO = lambda oi, ii: w(4 + oi * 2 + ii)    # noqa: E731
        W_FIRST = lambda oi: w(8 + oi)                # noqa: E731

        # chunk list: (dram_x_view, dram_y_view, chunk_idx_within_seq)
        chunks = []
        for src, dst in ((lo, out_lo), (up, out_up)):
            for b in range(B_PER_CORE):
                xv = src[b].rearrange("(J p i) d -> p J i d", p=P, i=ROWS)
                yv = dst[b].rearrange("(J p i) d -> p J i d", p=P, i=ROWS)
                for c in range(CHUNKS_PER_SEQ):
                    chunks.append((xv, yv, c))

        def emit_in(q):
            xv, _yv, c = chunks[q]
            J0 = c * CHUNK_TILES
            xt = x_pool.tile([P, CHUNK_COLS], f32)
            if q <= 1:
                # split the first load so the first exp can start sooner
                for h in (0, 1):
                    nc.sync.dma_start(
                        view3(xt[:])[:, h * HALF : (h + 1) * HALF, :, :],
                        xv[:, J0 + h * HALF : J0 + (h + 1) * HALF, :, :],
                    )
            else:
                nc.sync.dma_start(view3(xt[:]), xv[:, J0 : J0 + CHUNK_TILES, :, :])
            xts.append(xt)

        exp_insts = {}
        ln_insts = {}

        def emit_exp(q):
            # cols [0, TILE_COLS) hold the previous tile (halo); the chunk's
            # 16 tiles follow. Halo matmuls then read one tile-shifted views
            # with no extra split at the chunk boundary.
            _xv, _yv, c = chunks[q]
            et = e_pool.tile([P, TILE_COLS + CHUNK_COLS], f32r)
            if c > 0:
                nc.vector.tensor_copy(
                    et[:, 0:TILE_COLS], ets[q - 1][:, CHUNK_COLS:]
                )
            if q <= 1:
                HC = HALF * TILE_COLS
                exp_insts[q] = [
                    nc.scalar.activation(
                        et[:, TILE_COLS : TILE_COLS + HC],
                        xts[q][:, 0:HC], AF.Exp, scale=-SCALE,
                    ).ins,
                    nc.scalar.activation(
                        et[:, TILE_COLS + HC :], xts[q][:, HC:],
                        AF.Exp, scale=-SCALE,
                    ).ins,
                ]
            else:
                exp_insts[q] = [
                    nc.scalar.activation(
                        et[:, TILE_COLS:], xts[q][:], AF.Exp, scale=-SCALE
                    ).ins
                ]
            ets.append(et)

        xts = []
        ets = []
        if mode == "grouped":
            nc.scalar.dma_start(w_all[:], w_all_d[:].bitcast(f32r))
            for q in range(N_CHUNKS):
                emit_in(q)

        # ---- phase B: compute, activation-grouped
        pss = [None] * N_CHUNKS
        ots = [None] * N_CHUNKS

        def emit_mms(q):
            _xv, _yv, c = chunks[q]
            et3 = view3(ets[q][:, TILE_COLS:])
            hl3 = view3(ets[q][:, 0:CHUNK_COLS])  # tile-shifted (halo) view
            ps = ps_pool.tile([P, CHUNK_COLS], f32)
            pss[q] = ps

            mms = []

            def out_ap(oi, j_lo, j_hi):
                return ps[:, oi * 1024 + j_lo * D : oi * 1024 + j_hi * D]

            for oi in (0, 1):
                for ii in (0, 1):
                    lh = W_IN(oi, ii)
                    for h in (0, 1):
                        mms.append((
                            (oi, h), lh,
                            et3[:, h * HALF : (h + 1) * HALF, ii, :],
                            out_ap(oi, h * HALF, (h + 1) * HALF),
                        ))
            for oi in (0, 1):
                for ii in (0, 1):
                    lh = W_HALO(oi, ii)
                    if c > 0:
                        mms.append((
                            (oi, 0), lh,
                            hl3[:, 0:HALF, ii, :],
                            out_ap(oi, 0, HALF),
                        ))
                    else:
                        # first chunk: no halo tile; tiles 0..6 feed outputs
                        # 1..7 (tile 0's pad handled by W_first below)
                        mms.append((
                            (oi, 0), lh,
                            et3[:, 0 : HALF - 1, ii, :],
                            out_ap(oi, 1, HALF),
                        ))
                    mms.append((
                        (oi, 1), lh,
                        hl3[:, HALF:CHUNK_TILES, ii, :],
                        out_ap(oi, HALF, CHUNK_TILES),
                    ))
            if c == 0:
                for oi in (0, 1):
                    mms.append((
                        (oi, 0), W_FIRST(oi),
                        et3[:, 0:1, 0, :],
                        out_ap(oi, 0, 1),
                    ))

            first_seen, last_idx = set(), {}
            for k, (bank, *_rest) in enumerate(mms):
                last_idx[bank] = k
            for k, (bank, lh, rhs, outp) in enumerate(mms):
                st = bank not in first_seen
                first_seen.add(bank)
                nc.tensor.matmul(outp, lh, rhs, start=st, stop=(last_idx[bank] == k))

        def emit_ln(q):
            ot = o_pool.tile([P, CHUNK_COLS], f32)
            ots[q] = ot
            # ps iterates (oi, J, d); ot memory layout is (J, i, d)
            ps4 = pss[q][:].rearrange("p (oi J d) -> p oi J d", oi=2, d=D)
            ot4 = ot[:].rearrange("p (J i d) -> p i J d", i=ROWS, d=D)
            if q >= N_CHUNKS - 2:
                ln_insts[q] = []
                for h in (0, 1):
                    sl = slice(h * HALF, (h + 1) * HALF)
                    ln_insts[q].append(
                        nc.scalar.activation(
                            ot4[:, :, sl, :], ps4[:, :, sl, :], AF.Ln
                        ).ins
                    )
                    nc.vector.tensor_scalar_mul(
                        ot[:, h * HALF * TILE_COLS : (h + 1) * HALF * TILE_COLS],
                        ot[:, h * HALF * TILE_COLS : (h + 1) * HALF * TILE_COLS],
                        -1.0 / SCALE,
                    )
            else:
                ln_insts[q] = [
                    nc.scalar.activation(ot4, pss[q][:], AF.Ln).ins
                ]
                nc.vector.tensor_scalar_mul(ot[:], ot[:], -1.0 / SCALE)

        out_insts = {}

        def emit_out(q, engine=None):
            _xv, yv, c = chunks[q]
            J0 = c * CHUNK_TILES
            eng = engine if engine is not None else nc.sync
            if q >= N_CHUNKS - 2:
                for h in (0, 1):
                    out_insts[q] = eng.dma_start(
                        yv[:, J0 + h * HALF : J0 + (h + 1) * HALF, :, :],
                        view3(ots[q][:])[:, h * HALF : (h + 1) * HALF, :, :],
                    ).ins
            else:
                out_insts[q] = eng.dma_start(
                    yv[:, J0 : J0 + CHUNK_TILES, :, :], view3(ots[q][:])
                ).ins

        if mode == "grouped":
            for g in range(0, N_CHUNKS, ACT_GROUP):
                grp = list(range(g, min(g + ACT_GROUP, N_CHUNKS)))
                for q in grp:
                    emit_exp(q)
                for q in grp:
                    emit_mms(q)
                for q in grp:
                    emit_ln(q)
                # pin ACT order within/between groups so Exp and Ln table
                # sets switch once per phase (4 loads total), not per chunk
                for q in grp:
                    for q2 in grp:
                        for li in ln_insts[q]:
                            for ei in exp_insts[q2]:
                                add_dep_helper(
                                    li, ei, sync=False,
                                    reason="act table grouping",
                                )
                if g > 0:
                    for q in grp:
                        for q2 in range(g - ACT_GROUP, g):
                            for ei in exp_insts[q]:
                                for li in ln_insts[q2]:
                                    add_dep_helper(
                                        ei, li, sync=False,
                                        reason="act table grouping",
                                    )
            for q in range(N_CHUNKS):
                emit_out(q)
        elif mode == "perchunk":
            nc.sync.dma_start(w_all[:], w_all_d[:].bitcast(f32r))
            for q in range(N_CHUNKS):
                emit_in(q)
                emit_exp(q)
                emit_mms(q)
                emit_ln(q)
                emit_out(q)
        elif mode == "insfirst":
            nc.sync.dma_start(w_all[:], w_all_d[:].bitcast(f32r))
            for q in range(N_CHUNKS):
                emit_in(q)
            for q in range(N_CHUNKS):
                emit_exp(q)
                emit_mms(q)
                emit_ln(q)
            for q in range(N_CHUNKS):
                emit_out(q)
        else:
            raise ValueError(mode)
    nc.compile()
    return nc


def _run(lower_trace, upper_trace, trace=False, mode="grouped", **spmd_kwargs):
    from concourse.bass_utils import run_bass_kernel_spmd

    lower_trace = np.ascontiguousarray(np.asarray(lower_trace, dtype=np.float32))
    upper_trace = np.ascontiguousarray(np.asarray(upper_trace, dtype=np.float32))
    assert lower_trace.shape == (B, T, D) and upper_trace.shape == (B, T, D)

    nc = _build_bass(mode=mode)
    in_maps = [
        {
            "lower": np.ascontiguousarray(lower_trace[i * B_PER_CORE : (i + 1) * B_PER_CORE]),
            "upper": np.ascontiguousarray(upper_trace[i * B_PER_CORE : (i + 1) * B_PER_CORE]),
        }
        for i in range(N_CORES)
    ]
    res = run_bass_kernel_spmd(
        nc, in_maps, core_ids=list(range(N_CORES)), trace=trace, **spmd_kwargs
    )
    out_lower = np.concatenate([r["out_lower"] for r in res.results], axis=0)
    out_upper = np.concatenate([r["out_upper"] for r in res.results], axis=0)
    return (out_lower, out_upper), res


def kernel(lower_trace, upper_trace):
    (out_lower, out_upper), _ = _run(lower_trace, upper_trace, trace=False)
    return out_lower, out_upper
